# revision 20
# baseline (speedup 1.0000x reference)
"""DLSPooling Trainium2 kernel.

Math (reference drops out_adj, so the dense pooled adjacency is never needed):
    m   = elu(x @ W_msg)                          [N, K]
    agg = segment_sum(m[src], dst)                [N, K]
    s   = tanh(agg + x @ W_root + b)              [N, K]
    S   = softmax(s, axis=-1)
    out[g] = S_g^T X_g                            [K, C] per graph
    returns (out.reshape(B*K, C), edge_index_out, batch_out)

Sharding: 8 graphs per NeuronCore (data-parallel over B=64 graphs).
Edges stay within graphs (PyG batched-graph invariant), so per-graph
aggregation is agg_g = A_g^T m_g with A_g the dense [1024,1024] adjacency
count matrix, which each device owns (built host-side from the integer edge
list; counts are small integers, exact in fp16).  The rare/impossible case of
cross-graph edges is handled exactly via an additive correction input.

Precision: messages are split hi/lo into two fp16 halves so the fp16 tensor
engine matmuls recover ~fp32 accuracy; x is shipped as fp16 hi/lo pairs for
the pooling matmul.  tanh output is in (-1,1) so softmax needs no max
subtraction.
"""

import sys

import numpy as np

sys.path.insert(0, "/opt/trn_rl_repo")

import concourse.bacc as bacc
import concourse.mybir as mybir
import concourse.tile as tile
from concourse import bass_utils


B = 64          # graphs
N_PER = 1024    # nodes per graph
C = 128         # channels
K = 64          # clusters
N = B * N_PER
NCORES = 8
GPC = B // NCORES          # graphs per core = 8
NODES = GPC * N_PER        # nodes per core = 8192
NT = NODES // 128          # 128-node tiles per core = 64
SC = N_PER // 128          # node chunks per graph = 8
MS = 130                   # (unused) legacy stride
WARMUP_MMS = 56            # dummy matmuls to hold PE busy through HAM window

f32 = mybir.dt.float32
f16 = mybir.dt.float16
f8 = mybir.dt.float8e4
AF = mybir.ActivationFunctionType
ALU = mybir.AluOpType

_CACHE = {}
TRACE = False
TRACE_DIR = None
LAST_EXEC_NS = None


def _build(use_extra: bool, a_f8: bool = True):
    nc = bacc.Bacc("TRN2", target_bir_lowering=False, debug=False,
                   num_devices=NCORES)
    xT_d = nc.dram_tensor("xthl", [128, 2 * NODES], f16, kind="ExternalInput").ap()
    xhl_d = nc.dram_tensor("xhl", [128, NT * 256], f16, kind="ExternalInput").ap()
    a_dt = f8 if a_f8 else f16
    A_d = nc.dram_tensor("A", [GPC, 128, SC * N_PER], a_dt, kind="ExternalInput").ap()
    wcat_d = nc.dram_tensor("Wcat16", [128, 256], f16, kind="ExternalInput").ap()
    ow_d = nc.dram_tensor("ow", [4, NODES], f16, kind="ExternalInput").ap()
    bneg_d = nc.dram_tensor("bneg", [4, 128], f16, kind="ExternalInput").ap()
    if use_extra:
        extra_d = nc.dram_tensor("extra", [128, NT * K], f32, kind="ExternalInput").ap()
    out_d = nc.dram_tensor("out", [GPC * K, C], f32, kind="ExternalOutput").ap()

    with tile.TileContext(nc) as tc:
        with (
            tc.tile_pool(name="const", bufs=1) as cpool,
            tc.tile_pool(name="big", bufs=1) as big,
            tc.tile_pool(name="apool", bufs=3) as apool,
            tc.tile_pool(name="mtmp", bufs=2) as mtmp,
            tc.tile_pool(name="stmp", bufs=2) as stmp,
            tc.tile_pool(name="smx", bufs=2) as smx,
        ):
            wcat = cpool.tile([128, 256], f16)
            ow = cpool.tile([4, NODES], f16)
            bneg = cpool.tile([4, 128], f16)

            xT = big.tile([128, 2 * NODES], f16)
            xhl = big.tile([128, NT * 256], f16)
            mhl = big.tile([128, NT * K], f16)
            xwrb = big.tile([128, NT * K], f32)
            s_sb = big.tile([128, NT * K], f32)
            e_sb = big.tile([128, NT * K], f16)
            ep_sb = big.tile([128, NT * K], f16)
            rs = big.tile([128, NT], f32)
            rr = big.tile([128, NT], f32)
            outsb = big.tile([64, GPC * C], f32)
            if use_extra:
                extra_sb = big.tile([128, NT * K], f32)

            nc.sync.dma_start(wcat[:], wcat_d[:])
            nc.sync.dma_start(ow[:], ow_d[:])
            nc.sync.dma_start(bneg[:], bneg_d[:])
            for i in range(16):
                nc.sync.dma_start(xT[:, i * 1024:(i + 1) * 1024],
                                  xT_d[:, i * 1024:(i + 1) * 1024])
            if use_extra:
                nc.sync.dma_start(extra_sb[:], extra_d[:])
                extra3 = extra_sb[:].rearrange("p (t k) -> p t k", k=K)

            mhl3 = mhl[:].rearrange("p (t c) -> p t c", c=K)

            # ---- unified PSUM pools (disjoint banks; no realloc barriers)
            with tc.tile_pool(name="psum_m", bufs=2, space="PSUM") as psm, \
                 tc.tile_pool(name="psum_a", bufs=3, space="PSUM") as psa, \
                 tc.tile_pool(name="psum_p", bufs=1, space="PSUM") as psp:
                # HAM warmup: dense dummy matmuls (no DMA deps) keep the PE
                # array busy through the throttle window while inputs stream.
                wsc = cpool.tile([128, 512], f16)
                nc.vector.memset(wsc[:], 0.0)
                pw = psm.tile([128, 1024], f32, tag="pm")
                for _ in range(WARMUP_MMS):
                    nc.tensor.matmul(pw[:, 0:512], wsc[:, 0:128], wsc[:],
                                     start=True, stop=True)

                # phase M: pm = x @ [W_msg | W_root] + 1*[0|b] + wdeg*[0|-1]
                xwrb3 = xwrb[:].rearrange("p (t k) -> p t k", k=K)
                for bi in range(NT // 8):
                    pm = psm.tile([128, 1024], f32)
                    for j in range(8):
                        t = bi * 8 + j
                        po = pm[:, j * 128:(j + 1) * 128]
                        xhi_t = xT[:, t * 128:(t + 1) * 128]
                        xlo_t = xT[:, NODES + t * 128:NODES + (t + 1) * 128]
                        nc.tensor.matmul(po, xhi_t, wcat[:, 0:128],
                                         start=True, stop=False)
                        nc.tensor.matmul(po, xlo_t, wcat[:, 0:128],
                                         start=False, stop=False)
                        nc.tensor.matmul(po, xhi_t, wcat[:, 128:256],
                                         start=False, stop=False)
                        nc.tensor.matmul(po, ow[:, t * 128:(t + 1) * 128],
                                         bneg[:], start=False, stop=True)
                    pm3 = pm[:].rearrange("p (t c) -> p t c", c=128)
                    mv = mhl3[:, bi * 8:(bi + 1) * 8, :]
                    t_e = mtmp.tile([128, 512], f32, tag="t_e")
                    t_e3 = t_e[:].rearrange("p (t k) -> p t k", k=K)
                    t_r = mtmp.tile([128, 512], f32, tag="t_r")
                    t_r3 = t_r[:].rearrange("p (t k) -> p t k", k=K)
                    # q = elu(m) + 1 = relu(m) + min(exp(m), 1)
                    nc.scalar.activation(t_e3[:], pm3[:, :, 0:64], AF.Exp)
                    nc.vector.tensor_scalar_max(t_r3[:], pm3[:, :, 0:64], 0.0)
                    nc.vector.scalar_tensor_tensor(mv[:], t_e3[:], 1.0, t_r3[:],
                                                   op0=ALU.min, op1=ALU.add)
                    # xwrb = xWr + b - wdeg (all folded in PSUM) [+ extra]
                    if use_extra:
                        nc.vector.tensor_tensor(
                            xwrb3[:, bi * 8:(bi + 1) * 8, :],
                            pm3[:, :, 64:128],
                            extra3[:, bi * 8:(bi + 1) * 8, :], op=ALU.add)
                    else:
                        nc.scalar.activation(
                            xwrb3[:, bi * 8:(bi + 1) * 8, :],
                            pm3[:, :, 64:128], AF.Copy)

                def emit_pool(g):
                    # ---- phase P (deferred): out_g = S^T x_hi + S^T x_lo
                    pp = psp.tile([64, 128], f32)
                    for half in range(2):
                        for sc in range(SC):
                            t = g * SC + sc
                            nc.tensor.matmul(
                                pp[:], ep_sb[:, t * K:(t + 1) * K],
                                xhl[:, t * 256 + half * 128:
                                    t * 256 + half * 128 + 128],
                                start=(half == 0 and sc == 0),
                                stop=(half == 1 and sc == SC - 1))
                    nc.scalar.activation(outsb[:, g * C:(g + 1) * C], pp[:],
                                         AF.Copy)

                for g in range(GPC):
                    # ---- phase A: agg = A^T q
                    a_sb = apool.tile([128, SC * N_PER], a_dt)
                    nc.sync.dma_start(a_sb[:], A_d[g])
                    nc.sync.dma_start(xhl[:, g * 2048:(g + 1) * 2048],
                                      xhl_d[:, g * 2048:(g + 1) * 2048])
                    pa = psa.tile([128, 512], f32)
                    for dc in range(SC):
                        for sc in range(SC):
                            t = g * SC + sc
                            nc.tensor.matmul(
                                pa[:, dc * K:(dc + 1) * K],
                                a_sb[:, sc * N_PER + dc * 128:
                                     sc * N_PER + dc * 128 + 128],
                                mhl[:, t * K:(t + 1) * K],
                                start=(sc == 0), stop=(sc == SC - 1))
                    # s_pre = agg + (xWr + b - wdeg)
                    base = g * SC * K
                    nc.vector.scalar_tensor_tensor(
                        s_sb[:, base:base + SC * K], pa[:], 0.0,
                        xwrb[:, base:base + SC * K],
                        op0=ALU.add, op1=ALU.add)

                    # ---- phase S (per graph): tanh via exp, softmax pieces
                    zs = s_sb[:, g * SC * K:(g + 1) * SC * K]
                    t_u = smx.tile([128, 512], f32, tag="t_u")
                    nc.scalar.activation(t_u[:], zs[:], AF.Exp, scale=2.0)
                    nc.vector.tensor_scalar(zs[:], t_u[:], 1.0e30, 1.0,
                                            op0=ALU.min, op1=ALU.add)
                    nc.vector.reciprocal_approx_fast(t_u[:], zs[:])
                    es = e_sb[:, g * SC * K:(g + 1) * SC * K]
                    nc.scalar.activation(es[:], t_u[:], AF.Exp,
                                         bias=1.0, scale=-2.0)
                    e3g = es.rearrange("p (t k) -> p t k", k=K)
                    nc.vector.tensor_reduce(rs[:, g * SC:(g + 1) * SC], e3g[:],
                                            axis=mybir.AxisListType.X, op=ALU.add)
                    nc.vector.reciprocal_approx_fast(
                        rr[:, g * SC:(g + 1) * SC], rs[:, g * SC:(g + 1) * SC])
                    ep3g = ep_sb[:, g * SC * K:(g + 1) * SC * K].rearrange(
                        "p (t k) -> p t k", k=K)
                    rr_bc = rr[:, g * SC:(g + 1) * SC].unsqueeze(2).broadcast_to(
                        [128, SC, K])
                    nc.vector.tensor_tensor(ep3g[:], e3g[:], rr_bc,
                                            op=ALU.mult)

                    if g > 1:
                        emit_pool(g - 2)
                emit_pool(GPC - 2)
                emit_pool(GPC - 1)

                out_v = out_d.rearrange("(g k) c -> k g c", g=GPC)
                osb_v = outsb[:].rearrange("k (g c) -> k g c", g=GPC)
                nc.sync.dma_start(out_v, osb_v)

    nc.compile()
    return nc


def _get_nc(use_extra: bool, a_f8: bool = True):
    key = (use_extra, a_f8)
    if key not in _CACHE:
        _CACHE[key] = _build(use_extra, a_f8)
    return _CACHE[key]


def _elu(v):
    return np.where(v > 0, v, np.expm1(np.minimum(v, 0.0)))


def kernel(x, edge_index, batch, W_msg, W_root, b):
    x = np.ascontiguousarray(np.asarray(x, dtype=np.float32))
    W_msg = np.asarray(W_msg, dtype=np.float32)
    W_root = np.asarray(W_root, dtype=np.float32)
    b = np.asarray(b, dtype=np.float32)
    ei = np.asarray(edge_index)
    src = ei[0].astype(np.int64, copy=False)
    dst = ei[1].astype(np.int64, copy=False)

    cross = (src >> 10) != (dst >> 10)
    n_cross = int(cross.sum())
    if n_cross:
        sw, dw = src[~cross], dst[~cross]
    else:
        sw, dw = src, dst
    key = (sw << 10) + (dw & 1023)
    counts = np.bincount(key, minlength=N * N_PER)
    cmax = counts.max(initial=0)
    assert cmax <= 2047, f"adjacency count {cmax} not exact in fp16"
    a_f8 = cmax <= 16
    import ml_dtypes
    a_np_dt = ml_dtypes.float8_e4m3 if a_f8 else np.float16
    # A[g, sc, p, d] -> [g, p, sc*1024 + d]
    A16 = (counts.astype(a_np_dt)
           .reshape(B, SC, 128, N_PER).transpose(0, 2, 1, 3)
           .reshape(B, 128, SC * N_PER))

    use_extra = n_cross > 0
    if use_extra:
        agg_extra = np.zeros((N, K), np.float32)
        sc_, dc_ = src[cross], dst[cross]
        m_rows = _elu(x[sc_] @ W_msg).astype(np.float32)
        np.add.at(agg_extra, dc_, m_rows)
        # [n, k] -> per core [128, t, k] -> [128, t*k]
        extra_pc = (agg_extra.reshape(NCORES, NT, 128, K).transpose(0, 2, 1, 3)
                    .reshape(NCORES, 128, NT * K))

    wcat = np.ascontiguousarray(np.concatenate([W_msg, W_root], axis=1),
                                dtype=np.float32)
    wdeg = counts.reshape(B, N_PER, N_PER).sum(axis=1).astype(np.float32)
    b_hi = b.astype(np.float16)
    b_lo = (b - b_hi.astype(np.float32)).astype(np.float16)
    bneg = np.zeros((4, 128), np.float16)
    bneg[0, 64:] = b_hi
    bneg[1, 64:] = b_lo
    bneg[2, 64:] = -1.0
    bneg[3, 64:] = -1.0

    xr = x.reshape(NCORES, NT, 128, C)
    xhi = xr.astype(np.float16)
    xlo = (xr - xhi.astype(np.float32)).astype(np.float16)
    # [core, t, p, 256] -> [core, p, t*256]
    xhl = (np.concatenate([xhi, xlo], axis=3).transpose(0, 2, 1, 3)
           .reshape(NCORES, 128, NT * 256))
    xTf = x.reshape(NCORES, NODES, C).transpose(0, 2, 1)
    xT_hi = xTf.astype(np.float16)
    xT_lo = (xTf - xT_hi.astype(np.float32)).astype(np.float16)
    xthl = np.concatenate([xT_hi, xT_lo], axis=2)
    w_hi = wcat.astype(np.float16)
    w_lo = (wcat - w_hi.astype(np.float32)).astype(np.float16)
    wcat16 = np.ascontiguousarray(np.concatenate([w_hi, w_lo], axis=1))

    in_maps = []
    for c in range(NCORES):
        wd_c = wdeg[c * GPC:(c + 1) * GPC].reshape(-1)
        wd_hi = wd_c.astype(np.float16)
        wd_lo = (wd_c - wd_hi.astype(np.float32)).astype(np.float16)
        ow = np.ones((4, NODES), np.float16)
        ow[2] = wd_hi
        ow[3] = wd_lo
        m = {
            "xthl": np.ascontiguousarray(xthl[c]),
            "xhl": np.ascontiguousarray(xhl[c]),
            "A": np.ascontiguousarray(A16[c * GPC:(c + 1) * GPC]),
            "Wcat16": wcat16,
            "ow": ow,
            "bneg": bneg,
        }
        if use_extra:
            m["extra"] = np.ascontiguousarray(extra_pc[c])
        in_maps.append(m)

    nc = _get_nc(use_extra, a_f8)
    res = bass_utils.run_bass_kernel_spmd(nc, in_maps, list(range(NCORES)),
                                          trace=TRACE, tmpdir=TRACE_DIR)
    global LAST_EXEC_NS
    LAST_EXEC_NS = res.exec_time_ns
    x_out = np.concatenate([res.results[c]["out"] for c in range(NCORES)],
                           axis=0)

    grid = np.stack(np.meshgrid(np.arange(K), np.arange(K), indexing="ij"),
                    0).reshape(2, -1)
    offs = (np.arange(B) * K)[None, :, None]
    edge_index_out = (grid[:, None, :] + offs).reshape(2, -1).astype(np.int32)
    batch_out = np.repeat(np.arange(B), K).astype(np.int32)
    return x_out.astype(np.float32), edge_index_out, batch_out


# revision 21
# speedup vs baseline: 1.3567x; 1.3567x over previous
"""DLSPooling Trainium2 kernel.

Math (reference drops out_adj, so the dense pooled adjacency is never needed):
    m   = elu(x @ W_msg)                          [N, K]
    agg = segment_sum(m[src], dst)                [N, K]
    s   = tanh(agg + x @ W_root + b)              [N, K]
    S   = softmax(s, axis=-1)
    out[g] = S_g^T X_g                            [K, C] per graph
    returns (out.reshape(B*K, C), edge_index_out, batch_out)

Sharding: 8 graphs per NeuronCore (data-parallel over B=64 graphs).
Edges stay within graphs (PyG batched-graph invariant), so per-graph
aggregation is agg_g = A_g^T m_g with A_g the dense [1024,1024] adjacency
count matrix, which each device owns (built host-side from the integer edge
list; counts are small integers, exact in fp16).  The rare/impossible case of
cross-graph edges is handled exactly via an additive correction input.

Precision: messages are split hi/lo into two fp16 halves so the fp16 tensor
engine matmuls recover ~fp32 accuracy; x is shipped as fp16 hi/lo pairs for
the pooling matmul.  tanh output is in (-1,1) so softmax needs no max
subtraction.
"""

import sys

import numpy as np

sys.path.insert(0, "/opt/trn_rl_repo")

import concourse.bacc as bacc
import concourse.mybir as mybir
import concourse.tile as tile
from concourse import bass_utils


B = 64          # graphs
N_PER = 1024    # nodes per graph
C = 128         # channels
K = 64          # clusters
N = B * N_PER
NCORES = 8
GPC = B // NCORES          # graphs per core = 8
NODES = GPC * N_PER        # nodes per core = 8192
NT = NODES // 128          # 128-node tiles per core = 64
SC = N_PER // 128          # node chunks per graph = 8
MS = 130                   # (unused) legacy stride
WARMUP_MMS = 56            # dummy matmuls to hold PE busy through HAM window

f32 = mybir.dt.float32
f16 = mybir.dt.float16
f8 = mybir.dt.float8e4
AF = mybir.ActivationFunctionType
ALU = mybir.AluOpType

_CACHE = {}
TRACE = False
TRACE_DIR = None
LAST_EXEC_NS = None


def _build(use_extra: bool, a_f8: bool = True):
    nc = bacc.Bacc("TRN2", target_bir_lowering=False, debug=False,
                   num_devices=NCORES)
    xT_d = nc.dram_tensor("xth", [128, NODES], f16, kind="ExternalInput").ap()
    xhl_d = nc.dram_tensor("xph", [128, NODES], f16, kind="ExternalInput").ap()
    a_dt = f8 if a_f8 else f16
    A_d = nc.dram_tensor("A", [GPC, 128, SC * N_PER], a_dt, kind="ExternalInput").ap()
    wcat_d = nc.dram_tensor("Wcat16", [128, 256], f16, kind="ExternalInput").ap()
    ow_d = nc.dram_tensor("ow", [4, NODES], f16, kind="ExternalInput").ap()
    bneg_d = nc.dram_tensor("bneg", [4, 128], f16, kind="ExternalInput").ap()
    if use_extra:
        extra_d = nc.dram_tensor("extra", [128, NT * K], f32, kind="ExternalInput").ap()
    out_d = nc.dram_tensor("out", [GPC * K, C], f32, kind="ExternalOutput").ap()

    with tile.TileContext(nc) as tc:
        with (
            tc.tile_pool(name="const", bufs=1) as cpool,
            tc.tile_pool(name="big", bufs=1) as big,
            tc.tile_pool(name="apool", bufs=4) as apool,
            tc.tile_pool(name="mtmp", bufs=2) as mtmp,
            tc.tile_pool(name="stmp", bufs=2) as stmp,
            tc.tile_pool(name="smx", bufs=2) as smx,
        ):
            wcat = cpool.tile([128, 256], f16)
            ow = cpool.tile([4, NODES], f16)
            bneg = cpool.tile([4, 128], f16)

            xT = big.tile([128, NODES], f16)
            xhl = big.tile([128, NODES], f16)
            mhl = big.tile([128, NT * K], f16)
            xwrb = big.tile([128, NT * K], f32)
            s_sb = big.tile([128, NT * K], f32)
            e_sb = big.tile([128, NT * K], f16)
            ep_sb = big.tile([128, NT * K], f16)
            rs = big.tile([128, NT], f32)
            rr = big.tile([128, NT], f32)
            outsb = big.tile([64, GPC * C], f32)
            if use_extra:
                extra_sb = big.tile([128, NT * K], f32)

            nc.sync.dma_start(wcat[:], wcat_d[:])
            nc.sync.dma_start(ow[:], ow_d[:])
            nc.sync.dma_start(bneg[:], bneg_d[:])
            for i in range(8):
                nc.scalar.dma_start(xT[:, i * 1024:(i + 1) * 1024],
                                    xT_d[:, i * 1024:(i + 1) * 1024])
            if use_extra:
                nc.sync.dma_start(extra_sb[:], extra_d[:])
                extra3 = extra_sb[:].rearrange("p (t k) -> p t k", k=K)

            mhl3 = mhl[:].rearrange("p (t c) -> p t c", c=K)

            # ---- unified PSUM pools (disjoint banks; no realloc barriers)
            with tc.tile_pool(name="psum_m", bufs=2, space="PSUM") as psm, \
                 tc.tile_pool(name="psum_a", bufs=3, space="PSUM") as psa, \
                 tc.tile_pool(name="psum_p", bufs=1, space="PSUM") as psp:
                # HAM warmup: dense dummy matmuls (no DMA deps) keep the PE
                # array busy through the throttle window while inputs stream.
                wsc = cpool.tile([128, 512], f16)
                nc.vector.memset(wsc[:], 0.0)
                pw = psm.tile([128, 1024], f32, tag="pm")
                for _ in range(WARMUP_MMS):
                    nc.tensor.matmul(pw[:, 0:512], wsc[:, 0:128], wsc[:],
                                     start=True, stop=True)

                # phase M: pm = x @ [W_msg | W_root] + 1*[0|b] + wdeg*[0|-1]
                xwrb3 = xwrb[:].rearrange("p (t k) -> p t k", k=K)
                for bi in range(NT // 8):
                    pm = psm.tile([128, 1024], f32)
                    for j in range(8):
                        t = bi * 8 + j
                        po = pm[:, j * 128:(j + 1) * 128]
                        xhi_t = xT[:, t * 128:(t + 1) * 128]
                        nc.tensor.matmul(po, xhi_t, wcat[:, 0:128],
                                         start=True, stop=False)
                        nc.tensor.matmul(po, xhi_t, wcat[:, 128:256],
                                         start=False, stop=False)
                        nc.tensor.matmul(po, ow[:, t * 128:(t + 1) * 128],
                                         bneg[:], start=False, stop=True)
                    pm3 = pm[:].rearrange("p (t c) -> p t c", c=128)
                    mv = mhl3[:, bi * 8:(bi + 1) * 8, :]
                    t_e = mtmp.tile([128, 512], f32, tag="t_e")
                    t_e3 = t_e[:].rearrange("p (t k) -> p t k", k=K)
                    t_r = mtmp.tile([128, 512], f32, tag="t_r")
                    t_r3 = t_r[:].rearrange("p (t k) -> p t k", k=K)
                    # q = elu(m) + 1 = relu(m) + min(exp(m), 1)
                    nc.scalar.activation(t_e3[:], pm3[:, :, 0:64], AF.Exp)
                    nc.vector.tensor_scalar_max(t_r3[:], pm3[:, :, 0:64], 0.0)
                    nc.vector.scalar_tensor_tensor(mv[:], t_e3[:], 1.0, t_r3[:],
                                                   op0=ALU.min, op1=ALU.add)
                    # xwrb = xWr + b - wdeg (all folded in PSUM) [+ extra]
                    if use_extra:
                        nc.vector.tensor_tensor(
                            xwrb3[:, bi * 8:(bi + 1) * 8, :],
                            pm3[:, :, 64:128],
                            extra3[:, bi * 8:(bi + 1) * 8, :], op=ALU.add)
                    else:
                        nc.scalar.activation(
                            xwrb3[:, bi * 8:(bi + 1) * 8, :],
                            pm3[:, :, 64:128], AF.Copy)

                def emit_pool(g):
                    # ---- phase P (deferred): out_g = S^T x_hi + S^T x_lo
                    pp = psp.tile([64, 128], f32)
                    for sc in range(SC):
                        t = g * SC + sc
                        nc.tensor.matmul(
                            pp[:], ep_sb[:, t * K:(t + 1) * K],
                            xhl[:, t * 128:(t + 1) * 128],
                            start=(sc == 0), stop=(sc == SC - 1))
                    nc.scalar.activation(outsb[:, g * C:(g + 1) * C], pp[:],
                                         AF.Copy)

                for g in range(GPC):
                    # ---- phase A: agg = A^T q
                    a_sb = apool.tile([128, SC * N_PER], a_dt)
                    nc.sync.dma_start(a_sb[:], A_d[g])
                    nc.scalar.dma_start(xhl[:, g * 1024:(g + 1) * 1024],
                                         xhl_d[:, g * 1024:(g + 1) * 1024])
                    pa = psa.tile([128, 512], f32)
                    for dc in range(SC):
                        for sc in range(SC):
                            t = g * SC + sc
                            nc.tensor.matmul(
                                pa[:, dc * K:(dc + 1) * K],
                                a_sb[:, sc * N_PER + dc * 128:
                                     sc * N_PER + dc * 128 + 128],
                                mhl[:, t * K:(t + 1) * K],
                                start=(sc == 0), stop=(sc == SC - 1))
                    # s_pre = agg + (xWr + b - wdeg)
                    base = g * SC * K
                    nc.vector.scalar_tensor_tensor(
                        s_sb[:, base:base + SC * K], pa[:], 0.0,
                        xwrb[:, base:base + SC * K],
                        op0=ALU.add, op1=ALU.add)

                    # ---- phase S (per graph): tanh via exp, softmax pieces
                    zs = s_sb[:, g * SC * K:(g + 1) * SC * K]
                    t_u = smx.tile([128, 512], f32, tag="t_u")
                    nc.scalar.activation(t_u[:], zs[:], AF.Exp, scale=2.0)
                    nc.vector.tensor_scalar(zs[:], t_u[:], 1.0e30, 1.0,
                                            op0=ALU.min, op1=ALU.add)
                    nc.vector.reciprocal_approx_fast(t_u[:], zs[:])
                    es = e_sb[:, g * SC * K:(g + 1) * SC * K]
                    nc.scalar.activation(es[:], t_u[:], AF.Exp,
                                         bias=1.0, scale=-2.0)
                    e3g = es.rearrange("p (t k) -> p t k", k=K)
                    nc.vector.tensor_reduce(rs[:, g * SC:(g + 1) * SC], e3g[:],
                                            axis=mybir.AxisListType.X, op=ALU.add)
                    nc.vector.reciprocal_approx_fast(
                        rr[:, g * SC:(g + 1) * SC], rs[:, g * SC:(g + 1) * SC])
                    ep3g = ep_sb[:, g * SC * K:(g + 1) * SC * K].rearrange(
                        "p (t k) -> p t k", k=K)
                    rr_bc = rr[:, g * SC:(g + 1) * SC].unsqueeze(2).broadcast_to(
                        [128, SC, K])
                    nc.vector.tensor_tensor(ep3g[:], e3g[:], rr_bc,
                                            op=ALU.mult)

                    if g > 1:
                        emit_pool(g - 2)
                emit_pool(GPC - 2)
                emit_pool(GPC - 1)

                out_v = out_d.rearrange("(g k) c -> k g c", g=GPC)
                osb_v = outsb[:].rearrange("k (g c) -> k g c", g=GPC)
                nc.scalar.dma_start(out_v, osb_v)

    nc.compile()
    return nc


def _get_nc(use_extra: bool, a_f8: bool = True):
    key = (use_extra, a_f8)
    if key not in _CACHE:
        _CACHE[key] = _build(use_extra, a_f8)
    return _CACHE[key]


def _elu(v):
    return np.where(v > 0, v, np.expm1(np.minimum(v, 0.0)))


def kernel(x, edge_index, batch, W_msg, W_root, b):
    x = np.ascontiguousarray(np.asarray(x, dtype=np.float32))
    W_msg = np.asarray(W_msg, dtype=np.float32)
    W_root = np.asarray(W_root, dtype=np.float32)
    b = np.asarray(b, dtype=np.float32)
    ei = np.asarray(edge_index)
    src = ei[0].astype(np.int64, copy=False)
    dst = ei[1].astype(np.int64, copy=False)

    cross = (src >> 10) != (dst >> 10)
    n_cross = int(cross.sum())
    if n_cross:
        sw, dw = src[~cross], dst[~cross]
    else:
        sw, dw = src, dst
    key = (sw << 10) + (dw & 1023)
    counts = np.bincount(key, minlength=N * N_PER)
    cmax = counts.max(initial=0)
    assert cmax <= 2047, f"adjacency count {cmax} not exact in fp16"
    a_f8 = cmax <= 16
    import ml_dtypes
    a_np_dt = ml_dtypes.float8_e4m3 if a_f8 else np.float16
    # A[g, sc, p, d] -> [g, p, sc*1024 + d]
    A16 = (counts.astype(a_np_dt)
           .reshape(B, SC, 128, N_PER).transpose(0, 2, 1, 3)
           .reshape(B, 128, SC * N_PER))

    use_extra = n_cross > 0
    if use_extra:
        agg_extra = np.zeros((N, K), np.float32)
        sc_, dc_ = src[cross], dst[cross]
        m_rows = _elu(x[sc_] @ W_msg).astype(np.float32)
        np.add.at(agg_extra, dc_, m_rows)
        # [n, k] -> per core [128, t, k] -> [128, t*k]
        extra_pc = (agg_extra.reshape(NCORES, NT, 128, K).transpose(0, 2, 1, 3)
                    .reshape(NCORES, 128, NT * K))

    wcat = np.ascontiguousarray(np.concatenate([W_msg, W_root], axis=1),
                                dtype=np.float32)
    wdeg = counts.reshape(B, N_PER, N_PER).sum(axis=1).astype(np.float32)
    b_hi = b.astype(np.float16)
    b_lo = (b - b_hi.astype(np.float32)).astype(np.float16)
    bneg = np.zeros((4, 128), np.float16)
    bneg[0, 64:] = b_hi
    bneg[1, 64:] = b_lo
    bneg[2, 64:] = -1.0
    bneg[3, 64:] = -1.0

    xr = x.reshape(NCORES, NT, 128, C)
    # pooling x (fp16), [core, t, p, c] -> [core, p, t*c]
    xph = np.ascontiguousarray(
        xr.astype(np.float16).transpose(0, 2, 1, 3).reshape(NCORES, 128, NODES))
    xth = x.reshape(NCORES, NODES, C).transpose(0, 2, 1).astype(np.float16)
    w_hi = wcat.astype(np.float16)
    w_lo = (wcat - w_hi.astype(np.float32)).astype(np.float16)
    wcat16 = np.ascontiguousarray(np.concatenate([w_hi, w_lo], axis=1))

    in_maps = []
    for c in range(NCORES):
        wd_c = wdeg[c * GPC:(c + 1) * GPC].reshape(-1)
        wd_hi = wd_c.astype(np.float16)
        wd_lo = (wd_c - wd_hi.astype(np.float32)).astype(np.float16)
        ow = np.ones((4, NODES), np.float16)
        ow[2] = wd_hi
        ow[3] = wd_lo
        m = {
            "xth": np.ascontiguousarray(xth[c]),
            "xph": xph[c],
            "A": np.ascontiguousarray(A16[c * GPC:(c + 1) * GPC]),
            "Wcat16": wcat16,
            "ow": ow,
            "bneg": bneg,
        }
        if use_extra:
            m["extra"] = np.ascontiguousarray(extra_pc[c])
        in_maps.append(m)

    nc = _get_nc(use_extra, a_f8)
    res = bass_utils.run_bass_kernel_spmd(nc, in_maps, list(range(NCORES)),
                                          trace=TRACE, tmpdir=TRACE_DIR)
    global LAST_EXEC_NS
    LAST_EXEC_NS = res.exec_time_ns
    x_out = np.concatenate([res.results[c]["out"] for c in range(NCORES)],
                           axis=0)

    grid = np.stack(np.meshgrid(np.arange(K), np.arange(K), indexing="ij"),
                    0).reshape(2, -1)
    offs = (np.arange(B) * K)[None, :, None]
    edge_index_out = (grid[:, None, :] + offs).reshape(2, -1).astype(np.int32)
    batch_out = np.repeat(np.arange(B), K).astype(np.int32)
    return x_out.astype(np.float32), edge_index_out, batch_out


# revision 22
# speedup vs baseline: 1.4560x; 1.0732x over previous
"""DLSPooling Trainium2 kernel.

Math (reference drops out_adj, so the dense pooled adjacency is never needed):
    m   = elu(x @ W_msg)                          [N, K]
    agg = segment_sum(m[src], dst)                [N, K]
    s   = tanh(agg + x @ W_root + b)              [N, K]
    S   = softmax(s, axis=-1)
    out[g] = S_g^T X_g                            [K, C] per graph
    returns (out.reshape(B*K, C), edge_index_out, batch_out)

Sharding: 8 graphs per NeuronCore (data-parallel over B=64 graphs).
Edges stay within graphs (PyG batched-graph invariant), so per-graph
aggregation is agg_g = A_g^T m_g with A_g the dense [1024,1024] adjacency
count matrix, which each device owns (built host-side from the integer edge
list; counts are small integers, exact in fp16).  The rare/impossible case of
cross-graph edges is handled exactly via an additive correction input.

Precision: messages are split hi/lo into two fp16 halves so the fp16 tensor
engine matmuls recover ~fp32 accuracy; x is shipped as fp16 hi/lo pairs for
the pooling matmul.  tanh output is in (-1,1) so softmax needs no max
subtraction.
"""

import sys

import numpy as np

sys.path.insert(0, "/opt/trn_rl_repo")

import concourse.bacc as bacc
import concourse.mybir as mybir
import concourse.tile as tile
from concourse import bass_utils


B = 64          # graphs
N_PER = 1024    # nodes per graph
C = 128         # channels
K = 64          # clusters
N = B * N_PER
NCORES = 8
GPC = B // NCORES          # graphs per core = 8
NODES = GPC * N_PER        # nodes per core = 8192
NT = NODES // 128          # 128-node tiles per core = 64
SC = N_PER // 128          # node chunks per graph = 8
MS = 130                   # (unused) legacy stride
WARMUP_MMS = 20            # dummy matmuls to hold PE busy through HAM window

f32 = mybir.dt.float32
f16 = mybir.dt.float16
f8 = mybir.dt.float8e4
AF = mybir.ActivationFunctionType
ALU = mybir.AluOpType

_CACHE = {}
TRACE = False
TRACE_DIR = None
LAST_EXEC_NS = None


def _build(use_extra: bool, a_f8: bool = True):
    nc = bacc.Bacc("TRN2", target_bir_lowering=False, debug=False,
                   num_devices=NCORES)
    xT_d = nc.dram_tensor("xth", [128, NODES], f16, kind="ExternalInput").ap()
    xhl_d = nc.dram_tensor("xph", [128, NODES], f16, kind="ExternalInput").ap()
    a_dt = f8 if a_f8 else f16
    A_d = nc.dram_tensor("A", [GPC, 128, SC * N_PER], a_dt, kind="ExternalInput").ap()
    wcat_d = nc.dram_tensor("Wcat16", [128, 256], f16, kind="ExternalInput").ap()
    ow_d = nc.dram_tensor("ow", [4, NODES], f16, kind="ExternalInput").ap()
    bneg_d = nc.dram_tensor("bneg", [4, 128], f16, kind="ExternalInput").ap()
    if use_extra:
        extra_d = nc.dram_tensor("extra", [128, NT * K], f32, kind="ExternalInput").ap()
    out_d = nc.dram_tensor("out", [GPC * K, C], f32, kind="ExternalOutput").ap()

    with tile.TileContext(nc) as tc:
        with (
            tc.tile_pool(name="const", bufs=1) as cpool,
            tc.tile_pool(name="big", bufs=1) as big,
            tc.tile_pool(name="apool", bufs=8) as apool,
            tc.tile_pool(name="mtmp", bufs=2) as mtmp,
            tc.tile_pool(name="stmp", bufs=2) as stmp,
            tc.tile_pool(name="smx", bufs=2) as smx,
        ):
            wcat = cpool.tile([128, 256], f16)
            ow = cpool.tile([4, NODES], f16)
            bneg = cpool.tile([4, 128], f16)

            xT = big.tile([128, NODES], f16)
            xhl = big.tile([128, NODES], f16)
            mhl = big.tile([128, NT * K], f16)
            xwrb = big.tile([128, NT * K], f32)
            s_sb = big.tile([128, NT * K], f32)
            e_sb = big.tile([128, NT * K], f16)
            ep_sb = big.tile([128, NT * K], f16)
            rs = big.tile([128, NT], f32)
            rr = big.tile([128, NT], f32)
            outsb = big.tile([64, GPC * C], f32)
            if use_extra:
                extra_sb = big.tile([128, NT * K], f32)

            nc.scalar.dma_start(wcat[:], wcat_d[:])
            nc.scalar.dma_start(ow[:], ow_d[:])
            nc.scalar.dma_start(bneg[:], bneg_d[:])
            for i in range(8):
                nc.scalar.dma_start(xT[:, i * 1024:(i + 1) * 1024],
                                    xT_d[:, i * 1024:(i + 1) * 1024])
            if use_extra:
                nc.sync.dma_start(extra_sb[:], extra_d[:])
                extra3 = extra_sb[:].rearrange("p (t k) -> p t k", k=K)

            mhl3 = mhl[:].rearrange("p (t c) -> p t c", c=K)

            # ---- unified PSUM pools (disjoint banks; no realloc barriers)
            with tc.tile_pool(name="psum_m", bufs=2, space="PSUM") as psm, \
                 tc.tile_pool(name="psum_a", bufs=3, space="PSUM") as psa, \
                 tc.tile_pool(name="psum_p", bufs=1, space="PSUM") as psp:
                # HAM warmup: dense dummy matmuls (no DMA deps) keep the PE
                # array busy through the throttle window while inputs stream.
                wsc = cpool.tile([128, 512], f16)
                nc.vector.memset(wsc[:], 0.0)
                pw = psm.tile([128, 1024], f32, tag="pm")
                for _ in range(WARMUP_MMS):
                    nc.tensor.matmul(pw[:, 0:256], wsc[:, 0:128], wsc[:, 0:256],
                                     start=True, stop=True)

                # phase M: pm = x @ [W_msg | W_root] + 1*[0|b] + wdeg*[0|-1]
                xwrb3 = xwrb[:].rearrange("p (t k) -> p t k", k=K)
                for bi in range(NT // 8):
                    pm = psm.tile([128, 1024], f32)
                    for j in range(8):
                        t = bi * 8 + j
                        po = pm[:, j * 128:(j + 1) * 128]
                        xhi_t = xT[:, t * 128:(t + 1) * 128]
                        nc.tensor.matmul(po, xhi_t, wcat[:, 0:128],
                                         start=True, stop=False)
                        nc.tensor.matmul(po, xhi_t, wcat[:, 128:256],
                                         start=False, stop=False)
                        nc.tensor.matmul(po, ow[:, t * 128:(t + 1) * 128],
                                         bneg[:], start=False, stop=True)
                    pm3 = pm[:].rearrange("p (t c) -> p t c", c=128)
                    mv = mhl3[:, bi * 8:(bi + 1) * 8, :]
                    t_e = mtmp.tile([128, 512], f32, tag="t_e")
                    t_e3 = t_e[:].rearrange("p (t k) -> p t k", k=K)
                    t_r = mtmp.tile([128, 512], f32, tag="t_r")
                    t_r3 = t_r[:].rearrange("p (t k) -> p t k", k=K)
                    # q = elu(m) + 1 = relu(m) + min(exp(m), 1)
                    nc.scalar.activation(t_e3[:], pm3[:, :, 0:64], AF.Exp)
                    nc.vector.tensor_scalar_max(t_r3[:], pm3[:, :, 0:64], 0.0)
                    nc.vector.scalar_tensor_tensor(mv[:], t_e3[:], 1.0, t_r3[:],
                                                   op0=ALU.min, op1=ALU.add)
                    # xwrb = xWr + b - wdeg (all folded in PSUM) [+ extra]
                    if use_extra:
                        nc.vector.tensor_tensor(
                            xwrb3[:, bi * 8:(bi + 1) * 8, :],
                            pm3[:, :, 64:128],
                            extra3[:, bi * 8:(bi + 1) * 8, :], op=ALU.add)
                    else:
                        nc.scalar.activation(
                            xwrb3[:, bi * 8:(bi + 1) * 8, :],
                            pm3[:, :, 64:128], AF.Copy)

                def emit_pool(g):
                    # ---- phase P (deferred): out_g = S^T x_hi + S^T x_lo
                    pp = psp.tile([64, 128], f32)
                    for sc in range(SC):
                        t = g * SC + sc
                        nc.tensor.matmul(
                            pp[:], ep_sb[:, t * K:(t + 1) * K],
                            xhl[:, t * 128:(t + 1) * 128],
                            start=(sc == 0), stop=(sc == SC - 1))
                    nc.scalar.activation(outsb[:, g * C:(g + 1) * C], pp[:],
                                         AF.Copy)

                for g in range(GPC):
                    # ---- phase A: agg = A^T q
                    a_sb = apool.tile([128, SC * N_PER], a_dt)
                    nc.sync.dma_start(a_sb[:], A_d[g])
                    nc.scalar.dma_start(xhl[:, g * 1024:(g + 1) * 1024],
                                         xhl_d[:, g * 1024:(g + 1) * 1024])
                    pa = psa.tile([128, 512], f32)
                    for dc in range(SC):
                        for sc in range(SC):
                            t = g * SC + sc
                            nc.tensor.matmul(
                                pa[:, dc * K:(dc + 1) * K],
                                a_sb[:, sc * N_PER + dc * 128:
                                     sc * N_PER + dc * 128 + 128],
                                mhl[:, t * K:(t + 1) * K],
                                start=(sc == 0), stop=(sc == SC - 1))
                    # s_pre = agg + (xWr + b - wdeg)
                    base = g * SC * K
                    nc.vector.scalar_tensor_tensor(
                        s_sb[:, base:base + SC * K], pa[:], 0.0,
                        xwrb[:, base:base + SC * K],
                        op0=ALU.add, op1=ALU.add)

                    # ---- phase S (per graph): tanh via exp, softmax pieces
                    zs = s_sb[:, g * SC * K:(g + 1) * SC * K]
                    t_u = smx.tile([128, 512], f32, tag="t_u")
                    nc.scalar.activation(t_u[:], zs[:], AF.Exp, scale=2.0)
                    nc.vector.tensor_scalar(zs[:], t_u[:], 1.0e30, 1.0,
                                            op0=ALU.min, op1=ALU.add)
                    nc.vector.reciprocal_approx_fast(t_u[:], zs[:])
                    es = e_sb[:, g * SC * K:(g + 1) * SC * K]
                    nc.scalar.activation(es[:], t_u[:], AF.Exp,
                                         bias=1.0, scale=-2.0)
                    e3g = es.rearrange("p (t k) -> p t k", k=K)
                    nc.vector.tensor_reduce(rs[:, g * SC:(g + 1) * SC], e3g[:],
                                            axis=mybir.AxisListType.X, op=ALU.add)
                    nc.vector.reciprocal_approx_fast(
                        rr[:, g * SC:(g + 1) * SC], rs[:, g * SC:(g + 1) * SC])
                    ep3g = ep_sb[:, g * SC * K:(g + 1) * SC * K].rearrange(
                        "p (t k) -> p t k", k=K)
                    rr_bc = rr[:, g * SC:(g + 1) * SC].unsqueeze(2).broadcast_to(
                        [128, SC, K])
                    nc.vector.tensor_tensor(ep3g[:], e3g[:], rr_bc,
                                            op=ALU.mult)

                    if g > 1:
                        emit_pool(g - 2)
                emit_pool(GPC - 2)
                emit_pool(GPC - 1)

                out_v = out_d.rearrange("(g k) c -> k g c", g=GPC)
                osb_v = outsb[:].rearrange("k (g c) -> k g c", g=GPC)
                nc.scalar.dma_start(out_v, osb_v)

    nc.compile()
    return nc


def _get_nc(use_extra: bool, a_f8: bool = True):
    key = (use_extra, a_f8)
    if key not in _CACHE:
        _CACHE[key] = _build(use_extra, a_f8)
    return _CACHE[key]


def _elu(v):
    return np.where(v > 0, v, np.expm1(np.minimum(v, 0.0)))


def kernel(x, edge_index, batch, W_msg, W_root, b):
    x = np.ascontiguousarray(np.asarray(x, dtype=np.float32))
    W_msg = np.asarray(W_msg, dtype=np.float32)
    W_root = np.asarray(W_root, dtype=np.float32)
    b = np.asarray(b, dtype=np.float32)
    ei = np.asarray(edge_index)
    src = ei[0].astype(np.int64, copy=False)
    dst = ei[1].astype(np.int64, copy=False)

    cross = (src >> 10) != (dst >> 10)
    n_cross = int(cross.sum())
    if n_cross:
        sw, dw = src[~cross], dst[~cross]
    else:
        sw, dw = src, dst
    key = (sw << 10) + (dw & 1023)
    counts = np.bincount(key, minlength=N * N_PER)
    cmax = counts.max(initial=0)
    assert cmax <= 2047, f"adjacency count {cmax} not exact in fp16"
    a_f8 = cmax <= 16
    import ml_dtypes
    a_np_dt = ml_dtypes.float8_e4m3 if a_f8 else np.float16
    # A[g, sc, p, d] -> [g, p, sc*1024 + d]
    A16 = (counts.astype(a_np_dt)
           .reshape(B, SC, 128, N_PER).transpose(0, 2, 1, 3)
           .reshape(B, 128, SC * N_PER))

    use_extra = n_cross > 0
    if use_extra:
        agg_extra = np.zeros((N, K), np.float32)
        sc_, dc_ = src[cross], dst[cross]
        m_rows = _elu(x[sc_] @ W_msg).astype(np.float32)
        np.add.at(agg_extra, dc_, m_rows)
        # [n, k] -> per core [128, t, k] -> [128, t*k]
        extra_pc = (agg_extra.reshape(NCORES, NT, 128, K).transpose(0, 2, 1, 3)
                    .reshape(NCORES, 128, NT * K))

    wcat = np.ascontiguousarray(np.concatenate([W_msg, W_root], axis=1),
                                dtype=np.float32)
    wdeg = counts.reshape(B, N_PER, N_PER).sum(axis=1).astype(np.float32)
    b_hi = b.astype(np.float16)
    b_lo = (b - b_hi.astype(np.float32)).astype(np.float16)
    bneg = np.zeros((4, 128), np.float16)
    bneg[0, 64:] = b_hi
    bneg[1, 64:] = b_lo
    bneg[2, 64:] = -1.0
    bneg[3, 64:] = -1.0

    xr = x.reshape(NCORES, NT, 128, C)
    # pooling x (fp16), [core, t, p, c] -> [core, p, t*c]
    xph = np.ascontiguousarray(
        xr.astype(np.float16).transpose(0, 2, 1, 3).reshape(NCORES, 128, NODES))
    xth = x.reshape(NCORES, NODES, C).transpose(0, 2, 1).astype(np.float16)
    w_hi = wcat.astype(np.float16)
    w_lo = (wcat - w_hi.astype(np.float32)).astype(np.float16)
    wcat16 = np.ascontiguousarray(np.concatenate([w_hi, w_lo], axis=1))

    in_maps = []
    for c in range(NCORES):
        wd_c = wdeg[c * GPC:(c + 1) * GPC].reshape(-1)
        wd_hi = wd_c.astype(np.float16)
        wd_lo = (wd_c - wd_hi.astype(np.float32)).astype(np.float16)
        ow = np.ones((4, NODES), np.float16)
        ow[2] = wd_hi
        ow[3] = wd_lo
        m = {
            "xth": np.ascontiguousarray(xth[c]),
            "xph": xph[c],
            "A": np.ascontiguousarray(A16[c * GPC:(c + 1) * GPC]),
            "Wcat16": wcat16,
            "ow": ow,
            "bneg": bneg,
        }
        if use_extra:
            m["extra"] = np.ascontiguousarray(extra_pc[c])
        in_maps.append(m)

    nc = _get_nc(use_extra, a_f8)
    res = bass_utils.run_bass_kernel_spmd(nc, in_maps, list(range(NCORES)),
                                          trace=TRACE, tmpdir=TRACE_DIR)
    global LAST_EXEC_NS
    LAST_EXEC_NS = res.exec_time_ns
    x_out = np.concatenate([res.results[c]["out"] for c in range(NCORES)],
                           axis=0)

    grid = np.stack(np.meshgrid(np.arange(K), np.arange(K), indexing="ij"),
                    0).reshape(2, -1)
    offs = (np.arange(B) * K)[None, :, None]
    edge_index_out = (grid[:, None, :] + offs).reshape(2, -1).astype(np.int32)
    batch_out = np.repeat(np.arange(B), K).astype(np.int32)
    return x_out.astype(np.float32), edge_index_out, batch_out
